# revision 31
# baseline (speedup 1.0000x reference)
"""AnomalyAttention Trainium2 Bass kernel.

Computes, per (b, h) pair:
  attn   = (Q @ K^T) / sqrt(E)                    [L, S]
  series = softmax(attn, axis=-1)                 [L, S]
  V      = series @ values                        [L, D]
  sig    = 3^(sigmoid(5*sigma)+1e-5) - 1          [L]  (broadcast to [L, S])
  prior  = 1/sqrt(2pi)/sig * exp(-|i-j|^2/(2 sig^2))   [L, S]

Sharding: 32 (b, h) pairs spread over 8 NeuronCores, 4 pairs each
(data parallel over B, tensor parallel over H). No cross-core comms.

Key implementation choices:
 - Q^T / K^T are pre-transposed on host so both matmul operands have the
   contraction dim (E=64) on partitions; scores [l,s] AND scores^T [s,l]
   are both computed on the TensorEngine (a transposed copy of exp(scores)
   is needed as the stationary operand of the series @ V matmul, and
   recomputing the matmul is the same PE cost as PE-transposing + copying).
 - softmax skips max-subtraction (scores ~ N(0,1), |score| << 80), so
   exp is a single fused ScalarE op (scale=1/8) whose accum_out gives the
   softmax denominators for free.
 - prior is a band matrix in fp32: sig <= 2, so exp underflows to exactly
   0 beyond |i-j| > 29. Only a 192-wide band per 128-row tile is computed
   (one fused ScalarE exp: Exp(d2 * (-1/(2 sig^2)) + ln(c/sig))) and
   written; the rest of the output buffer stays zero (output buffers are
   zero-initialized by the runtime).
"""

import math
import sys

sys.path.insert(0, "/opt/trn_rl_repo")

import numpy as np

import concourse.bacc as bacc
import concourse.mybir as mybir
import concourse.tile as tile
from concourse.bass_utils import run_bass_kernel_spmd

B, L, H, E = 4, 1024, 8, 64
NPAIR = 4  # (b, h) pairs per core
NT = 8  # 128-row tiles per L
BAND = 192  # prior band width per row-tile
N_CORES = 8

_LN3 = math.log(3.0)
_CNORM = 1.0 / math.sqrt(2.0 * math.pi)
_SCALE = 1.0 / math.sqrt(E)

F32 = mybir.dt.float32
BF16 = mybir.dt.bfloat16
AF = mybir.ActivationFunctionType
ALU = mybir.AluOpType


def _band_c0(j):
    # leftmost column of the 192-wide prior band for row-tile j
    if j == 0:
        return 0
    if j == NT - 1:
        return L - BAND
    return j * 128 - 32


def _d2_band():
    """d2[p, v, s'] = (i - j)^2 restricted to the band, for the 3 distinct
    band offsets (first tile, interior tiles, last tile)."""
    p = np.arange(128, dtype=np.float32)[:, None]
    s = np.arange(BAND, dtype=np.float32)[None, :]
    out = np.empty((128, 3, BAND), dtype=np.float32)
    out[:, 0, :] = (p - s) ** 2  # j=0:    c0 = 0,   l - s' = p - s'
    out[:, 1, :] = (p + 32 - s) ** 2  # 0<j<7:  c0 = 128j-32
    out[:, 2, :] = (p + 64 - s) ** 2  # j=7:    c0 = 832 (clamped)
    return out


def build_nc(n_pairs=NPAIR):
    nc = bacc.Bacc("TRN2", target_bir_lowering=False, debug=False, num_devices=N_CORES)

    # matmul operands are bf16 (fp32 matmul streams at 1/4 rate and is
    # split into hi/lo passes by the compiler — 8x the PE time)
    qT = nc.declare_dram_parameter("qT", [n_pairs, E, L], BF16, isOutput=False)
    kT = nc.declare_dram_parameter("kT", [n_pairs, E, L], BF16, isOutput=False)
    v = nc.declare_dram_parameter("v", [n_pairs, L, E], BF16, isOutput=False)
    sg = nc.declare_dram_parameter("sg", [128, n_pairs, NT], F32, isOutput=False)
    d2b = nc.declare_dram_parameter("d2b", [128, 3, BAND], F32, isOutput=False)

    oV = nc.declare_dram_parameter("oV", [n_pairs, L, E], F32, isOutput=True)
    oS = nc.declare_dram_parameter("oS", [n_pairs, L, L], F32, isOutput=True)
    oP = nc.declare_dram_parameter("oP", [n_pairs, L, L], F32, isOutput=True)
    oG = nc.declare_dram_parameter("oG", [n_pairs, L, L], F32, isOutput=True)

    from contextlib import ExitStack

    with tile.TileContext(nc) as tc, ExitStack() as ctx:
        const = ctx.enter_context(tc.tile_pool(name="const", bufs=1))
        ones_t = const.tile([128, L], F32)
        nc.vector.memset(ones_t, 1.0)
        d2b_t = const.tile([128, 3, BAND], F32)
        nc.sync.dma_start(out=d2b_t, in_=d2b[:, :, :])

        qk_pool = ctx.enter_context(tc.tile_pool(name="qk", bufs=2))
        v_pool = ctx.enter_context(tc.tile_pool(name="vp", bufs=2))
        small = ctx.enter_context(tc.tile_pool(name="small", bufs=2))
        expT_pool = ctx.enter_context(tc.tile_pool(name="expT", bufs=2))
        ser_pool = ctx.enter_context(tc.tile_pool(name="ser", bufs=4))
        sgbc_pool = ctx.enter_context(tc.tile_pool(name="sgbc", bufs=4))
        pr_pool = ctx.enter_context(tc.tile_pool(name="pr", bufs=4))
        vout_pool = ctx.enter_context(tc.tile_pool(name="vout", bufs=4))
        psum_sc = ctx.enter_context(tc.tile_pool(name="psc", bufs=3, space="PSUM"))
        psum_v = ctx.enter_context(tc.tile_pool(name="psv", bufs=2, space="PSUM"))

        # --- sigma path for ALL pairs at once (only depends on sigma):
        # makes every pair's prior/sig_bc DMA-ready early so those streams
        # can fill DMA-queue bubbles throughout the kernel ---
        W = n_pairs * NT
        sgm = small.tile([128, n_pairs, NT], F32, tag="sgm")
        nc.sync.dma_start(out=sgm, in_=sg[:, :, :])
        sgm_f = sgm.rearrange("p a b -> p (a b)")
        # s = sigmoid(5x) via explicit e^z/(1+e^z): the ACT exp table has
        # ~1e-5 *relative* error, so this keeps s accurate at the tails
        # (the Sigmoid table's absolute error there would be amplified
        # ~1e5x by the downstream 1/sig).
        ez = small.tile([128, W], F32, tag="ez")
        nc.scalar.activation(ez, sgm_f, AF.Exp, scale=5.0)
        ez1 = small.tile([128, W], F32, tag="ez1")
        nc.vector.tensor_scalar_add(ez1, ez, 1.0)
        rz = small.tile([128, W], F32, tag="rz")
        nc.vector.reciprocal(rz, ez1)
        s_t = small.tile([128, W], F32, tag="s_t")
        nc.vector.tensor_mul(s_t, ez, rz)
        # t = ln3 * (s + 1e-5);  sig = expm1(t) = 3^(s+1e-5) - 1
        t_t = small.tile([128, W], F32, tag="t_t")
        nc.vector.tensor_scalar(t_t, s_t, 1e-5, _LN3, ALU.add, ALU.mult)
        # big-t branch: exp(t) - 1
        eb = small.tile([128, W], F32, tag="eb")
        nc.scalar.activation(eb, t_t, AF.Exp)
        nc.vector.tensor_scalar_add(eb, eb, -1.0)
        # small-t branch: t + t^2/2 + t^3/6 (avoids the catastrophic
        # cancellation of exp(t)-1 for t ~ 1e-5)
        pa = small.tile([128, W], F32, tag="pa")
        nc.vector.tensor_scalar(pa, t_t, 1.0 / 3.0, 1.0, ALU.mult, ALU.add)
        nc.vector.tensor_mul(pa, t_t, pa)
        nc.vector.tensor_scalar(pa, pa, 0.5, 1.0, ALU.mult, ALU.add)
        nc.vector.tensor_mul(pa, t_t, pa)
        mk = small.tile([128, W], mybir.dt.uint8, tag="mk")
        nc.vector.tensor_scalar(mk, t_t, 0.1, None, ALU.is_lt)
        sig_a = small.tile([128, n_pairs, NT], F32, tag="sig")
        sig_f = sig_a.rearrange("p a b -> p (a b)")
        nc.vector.select(sig_f, mk, pa, eb)

        s2 = small.tile([128, W], F32, tag="s2")
        nc.vector.tensor_mul(s2, sig_f, sig_f)
        ni_a = small.tile([128, n_pairs, NT], F32, tag="ni")
        ni_f = ni_a.rearrange("p a b -> p (a b)")
        nc.vector.reciprocal(ni_f, s2)
        nc.vector.tensor_scalar_mul(ni_f, ni_f, -0.5)  # -1/(2 sig^2)
        # c/sig as a DVE post-multiply (an exp bias of ln(c/sig) would
        # need the Ln table -> ~2.7us ACT table switches per swap)
        cs_a = small.tile([128, n_pairs, NT], F32, tag="cs")
        cs_f = cs_a.rearrange("p a b -> p (a b)")
        nc.vector.reciprocal(cs_f, sig_f)
        nc.vector.tensor_scalar_mul(cs_f, cs_f, _CNORM)

        vo_tiles = []
        for pr in range(n_pairs):
            qt = qk_pool.tile([E, L], BF16, tag="qt")
            nc.sync.dma_start(out=qt, in_=qT[pr])
            kt = qk_pool.tile([E, L], BF16, tag="kt")
            nc.sync.dma_start(out=kt, in_=kT[pr])
            vt = v_pool.tile([128, NT, E], BF16, tag="vt")
            nc.sync.dma_start(out=vt, in_=v[pr].rearrange("(j p) e -> p j e", p=128))
            # --- sig broadcast [L] -> [L, S] (DVE compute; DMAs ride the
            # SWDGE queues to keep descriptor-gen off the sync sequencer) ---
            for j2 in range(NT // 2):
                gb = sgbc_pool.tile([128, 2, L], F32, tag="gb")
                for jj in range(2):
                    j = j2 * 2 + jj
                    nc.vector.tensor_scalar_mul(
                        gb[:, jj, :], ones_t, sig_a[:, pr, j : j + 1]
                    )
                nc.gpsimd.dma_start(
                    out=oG[pr, j2 * 256 : (j2 + 1) * 256, :].rearrange(
                        "(jj p) s -> p jj s", p=128
                    ),
                    in_=gb,
                )
            # --- prior band: (c/sig) * Exp(d2 * (-1/(2 sig^2))) ---
            for j in range(NT):
                vt_i = 0 if j == 0 else (2 if j == NT - 1 else 1)
                c0 = _band_c0(j)
                pb = pr_pool.tile([128, BAND], F32, tag="pb")
                nc.scalar.activation(
                    pb, d2b_t[:, vt_i, :], AF.Exp, scale=ni_a[:, pr, j : j + 1]
                )
                nc.vector.tensor_scalar_mul(pb, pb, cs_a[:, pr, j : j + 1])
                nc.gpsimd.dma_start(
                    out=oP[pr, j * 128 : (j + 1) * 128, c0 : c0 + BAND], in_=pb
                )
            # --- scores, exp, softmax denominators, series out ---
            # (2 row-tiles per SBUF tile -> 1MB DMAs, half the descriptor-gen
            # work on the sync sequencer)
            den = small.tile([128, NT], F32, tag="den")
            rden = small.tile([128, NT], F32, tag="rden")
            for j2 in range(NT // 2):
                es2 = ser_pool.tile([128, 2, L], F32, tag="es")
                for jj in range(2):
                    j = j2 * 2 + jj
                    ps = psum_sc.tile([128, L], F32, tag="scp")
                    nc.tensor.matmul(
                        ps[:, 0:512], qt[:, j * 128 : (j + 1) * 128], kt[:, 0:512],
                        start=True, stop=True,
                    )
                    nc.tensor.matmul(
                        ps[:, 512:1024], qt[:, j * 128 : (j + 1) * 128],
                        kt[:, 512:1024], start=True, stop=True,
                    )
                    nc.scalar.activation(
                        es2[:, jj, :], ps, AF.Exp, scale=_SCALE,
                        accum_out=den[:, j : j + 1],
                    )
                    nc.vector.reciprocal(rden[:, j : j + 1], den[:, j : j + 1])
                    nc.vector.tensor_scalar_mul(
                        es2[:, jj, :], es2[:, jj, :], rden[:, j : j + 1]
                    )
                nc.sync.dma_start(
                    out=oS[pr, j2 * 256 : (j2 + 1) * 256, :].rearrange(
                        "(jj p) s -> p jj s", p=128
                    ),
                    in_=es2,
                )

            # --- scores^T, exp (stationary operand for the AV matmul) ---
            exT = expT_pool.tile([128, NT, L], BF16, tag="exT")
            for i in range(NT):
                ps = psum_sc.tile([128, L], F32, tag="scp")
                nc.tensor.matmul(
                    ps[:, 0:512], kt[:, i * 128 : (i + 1) * 128], qt[:, 0:512],
                    start=True, stop=True,
                )
                nc.tensor.matmul(
                    ps[:, 512:1024], kt[:, i * 128 : (i + 1) * 128], qt[:, 512:1024],
                    start=True, stop=True,
                )
                nc.scalar.activation(exT[:, i, :], ps, AF.Exp, scale=_SCALE)

            # --- V = series @ values (accumulate over s-tiles), normalize.
            # The DMA is deferred to after the pair loop: a V write here would
            # sit in the gpsimd sequencer program ahead of the next pair's
            # sig_bc/prior triggers and stall that whole DMA stream on the
            # AV matmul chain. ---
            vo = vout_pool.tile([128, NT, E], F32, tag="vo")
            vo_tiles.append(vo)
            for j in range(NT):
                pv = psum_v.tile([128, E], F32, tag="pv")
                for i in range(NT):
                    nc.tensor.matmul(
                        pv,
                        exT[:, i, j * 128 : (j + 1) * 128],
                        vt[:, i, :],
                        start=(i == 0),
                        stop=(i == NT - 1),
                    )
                nc.vector.tensor_scalar_mul(vo[:, j, :], pv, rden[:, j : j + 1])



        for pr, vo in enumerate(vo_tiles):
            nc.gpsimd.dma_start(
                out=oV[pr].rearrange("(j p) e -> p j e", p=128), in_=vo
            )

    nc.compile()
    return nc


_NC = None


def _get_nc():
    global _NC
    if _NC is None:
        _NC = build_nc()
    return _NC


def _pairs(core):
    return [(f // H, f % H) for f in range(core * NPAIR, (core + 1) * NPAIR)]


def _make_in_maps(inputs):
    from ml_dtypes import bfloat16

    queries = np.asarray(inputs["queries"], dtype=np.float32)
    keys = np.asarray(inputs["keys"], dtype=np.float32)
    values = np.asarray(inputs["values"], dtype=np.float32)
    sigma = np.asarray(inputs["sigma"], dtype=np.float32)

    d2b = _d2_band()
    in_maps = []
    for c in range(N_CORES):
        prs = _pairs(c)
        in_maps.append(
            {
                "qT": np.stack(
                    [queries[b, :, h, :].T for b, h in prs]
                ).astype(bfloat16),
                "kT": np.stack([keys[b, :, h, :].T for b, h in prs]).astype(bfloat16),
                "v": np.stack([values[b, :, h, :] for b, h in prs]).astype(bfloat16),
                "sg": np.stack(
                    [sigma[b, :, h].reshape(NT, 128).T for b, h in prs], axis=1
                ).copy(),
                "d2b": d2b,
            }
        )
    return in_maps


def kernel(queries, keys, values, sigma):
    nc = _get_nc()
    in_maps = _make_in_maps(
        {"queries": queries, "keys": keys, "values": values, "sigma": sigma}
    )

    res = run_bass_kernel_spmd(nc, in_maps, core_ids=list(range(N_CORES)))

    V = np.empty((B, L, H, E), np.float32)
    series = np.empty((B, H, L, L), np.float32)
    prior = np.empty((B, H, L, L), np.float32)
    sigbc = np.empty((B, H, L, L), np.float32)
    for c in range(N_CORES):
        r = res.results[c]
        for idx, (b, h) in enumerate(_pairs(c)):
            V[b, :, h, :] = r["oV"][idx]
            series[b, h] = r["oS"][idx]
            prior[b, h] = r["oP"][idx]
            sigbc[b, h] = r["oG"][idx]
    return V, series, prior, sigbc


# revision 32
# speedup vs baseline: 1.1394x; 1.1394x over previous
"""AnomalyAttention Trainium2 Bass kernel.

Computes, per (b, h) pair:
  attn   = (Q @ K^T) / sqrt(E)                    [L, S]
  series = softmax(attn, axis=-1)                 [L, S]
  V      = series @ values                        [L, D]
  sig    = 3^(sigmoid(5*sigma)+1e-5) - 1          [L]  (broadcast to [L, S])
  prior  = 1/sqrt(2pi)/sig * exp(-|i-j|^2/(2 sig^2))   [L, S]

Sharding: 32 (b, h) pairs spread over 8 NeuronCores, 4 pairs each
(data parallel over B, tensor parallel over H). No cross-core comms.

Key implementation choices:
 - Q^T / K^T are pre-transposed on host so both matmul operands have the
   contraction dim (E=64) on partitions; scores [l,s] AND scores^T [s,l]
   are both computed on the TensorEngine (a transposed copy of exp(scores)
   is needed as the stationary operand of the series @ V matmul, and
   recomputing the matmul is the same PE cost as PE-transposing + copying).
 - softmax skips max-subtraction (scores ~ N(0,1), |score| << 80), so
   exp is a single fused ScalarE op (scale=1/8) whose accum_out gives the
   softmax denominators for free.
 - prior is a band matrix in fp32: sig <= 2, so exp underflows to exactly
   0 beyond |i-j| > 29. Only a 192-wide band per 128-row tile is computed
   (one fused ScalarE exp: Exp(d2 * (-1/(2 sig^2)) + ln(c/sig))) and
   written; the rest of the output buffer stays zero (output buffers are
   zero-initialized by the runtime).
"""

import math
import sys

sys.path.insert(0, "/opt/trn_rl_repo")

import numpy as np

import concourse.bacc as bacc
import concourse.mybir as mybir
import concourse.tile as tile
from concourse.bass_utils import run_bass_kernel_spmd

B, L, H, E = 4, 1024, 8, 64
NPAIR = 4  # (b, h) pairs per core
NT = 8  # 128-row tiles per L
BAND = 192  # prior band width per row-tile
N_CORES = 8

_LN3 = math.log(3.0)
_CNORM = 1.0 / math.sqrt(2.0 * math.pi)
_SCALE = 1.0 / math.sqrt(E)

F32 = mybir.dt.float32
BF16 = mybir.dt.bfloat16
AF = mybir.ActivationFunctionType
ALU = mybir.AluOpType


def _band_c0(j):
    # leftmost column of the 192-wide prior band for row-tile j
    if j == 0:
        return 0
    if j == NT - 1:
        return L - BAND
    return j * 128 - 32


def _d2_band():
    """d2[p, v, s'] = (i - j)^2 restricted to the band, for the 3 distinct
    band offsets (first tile, interior tiles, last tile)."""
    p = np.arange(128, dtype=np.float32)[:, None]
    s = np.arange(BAND, dtype=np.float32)[None, :]
    out = np.empty((128, 3, BAND), dtype=np.float32)
    out[:, 0, :] = (p - s) ** 2  # j=0:    c0 = 0,   l - s' = p - s'
    out[:, 1, :] = (p + 32 - s) ** 2  # 0<j<7:  c0 = 128j-32
    out[:, 2, :] = (p + 64 - s) ** 2  # j=7:    c0 = 832 (clamped)
    return out


def build_nc(n_pairs=NPAIR):
    nc = bacc.Bacc("TRN2", target_bir_lowering=False, debug=False, num_devices=N_CORES)

    # matmul operands are bf16 (fp32 matmul streams at 1/4 rate and is
    # split into hi/lo passes by the compiler — 8x the PE time)
    qT = nc.declare_dram_parameter("qT", [n_pairs, E, L], BF16, isOutput=False)
    kT = nc.declare_dram_parameter("kT", [n_pairs, E, L], BF16, isOutput=False)
    v = nc.declare_dram_parameter("v", [n_pairs, L, E], BF16, isOutput=False)
    sg = nc.declare_dram_parameter("sg", [128, n_pairs, NT], F32, isOutput=False)
    d2b = nc.declare_dram_parameter("d2b", [128, 3, BAND], F32, isOutput=False)

    oV = nc.declare_dram_parameter("oV", [n_pairs, L, E], F32, isOutput=True)
    oS = nc.declare_dram_parameter("oS", [n_pairs, L, L], F32, isOutput=True)
    oP = nc.declare_dram_parameter("oP", [n_pairs, L, L], F32, isOutput=True)
    oG = nc.declare_dram_parameter("oG", [n_pairs, L, L], F32, isOutput=True)

    from contextlib import ExitStack

    with tile.TileContext(nc) as tc, ExitStack() as ctx:
        const = ctx.enter_context(tc.tile_pool(name="const", bufs=1))
        ones_t = const.tile([128, L], F32)
        nc.vector.memset(ones_t, 1.0)
        d2b_t = const.tile([128, 3, BAND], F32)
        nc.sync.dma_start(out=d2b_t, in_=d2b[:, :, :])

        qk_pool = ctx.enter_context(tc.tile_pool(name="qk", bufs=2))
        v_pool = ctx.enter_context(tc.tile_pool(name="vp", bufs=2))
        small = ctx.enter_context(tc.tile_pool(name="small", bufs=2))
        expT_pool = ctx.enter_context(tc.tile_pool(name="expT", bufs=2))
        ser_pool = ctx.enter_context(tc.tile_pool(name="ser", bufs=4))
        sgbc_pool = ctx.enter_context(tc.tile_pool(name="sgbc", bufs=4))
        pr_pool = ctx.enter_context(tc.tile_pool(name="pr", bufs=4))
        vout_pool = ctx.enter_context(tc.tile_pool(name="vout", bufs=4))
        psum_sc = ctx.enter_context(tc.tile_pool(name="psc", bufs=3, space="PSUM"))
        psum_v = ctx.enter_context(tc.tile_pool(name="psv", bufs=2, space="PSUM"))

        # --- sigma path for ALL pairs at once (only depends on sigma):
        # makes every pair's prior/sig_bc DMA-ready early so those streams
        # can fill DMA-queue bubbles throughout the kernel ---
        W = n_pairs * NT
        sgm = small.tile([128, n_pairs, NT], F32, tag="sgm")
        nc.sync.dma_start(out=sgm, in_=sg[:, :, :])
        sgm_f = sgm.rearrange("p a b -> p (a b)")
        # s = sigmoid(5x) via explicit e^z/(1+e^z): the ACT exp table has
        # ~1e-5 *relative* error, so this keeps s accurate at the tails
        # (the Sigmoid table's absolute error there would be amplified
        # ~1e5x by the downstream 1/sig).
        ez = small.tile([128, W], F32, tag="ez")
        nc.scalar.activation(ez, sgm_f, AF.Exp, scale=5.0)
        ez1 = small.tile([128, W], F32, tag="ez1")
        nc.vector.tensor_scalar_add(ez1, ez, 1.0)
        rz = small.tile([128, W], F32, tag="rz")
        nc.vector.reciprocal(rz, ez1)
        s_t = small.tile([128, W], F32, tag="s_t")
        nc.vector.tensor_mul(s_t, ez, rz)
        # t = ln3 * (s + 1e-5);  sig = expm1(t) = 3^(s+1e-5) - 1
        t_t = small.tile([128, W], F32, tag="t_t")
        nc.vector.tensor_scalar(t_t, s_t, 1e-5, _LN3, ALU.add, ALU.mult)
        # big-t branch: exp(t) - 1
        eb = small.tile([128, W], F32, tag="eb")
        nc.scalar.activation(eb, t_t, AF.Exp)
        nc.vector.tensor_scalar_add(eb, eb, -1.0)
        # small-t branch: t + t^2/2 + t^3/6 (avoids the catastrophic
        # cancellation of exp(t)-1 for t ~ 1e-5)
        pa = small.tile([128, W], F32, tag="pa")
        nc.vector.tensor_scalar(pa, t_t, 1.0 / 3.0, 1.0, ALU.mult, ALU.add)
        nc.vector.tensor_mul(pa, t_t, pa)
        nc.vector.tensor_scalar(pa, pa, 0.5, 1.0, ALU.mult, ALU.add)
        nc.vector.tensor_mul(pa, t_t, pa)
        mk = small.tile([128, W], mybir.dt.uint8, tag="mk")
        nc.vector.tensor_scalar(mk, t_t, 0.1, None, ALU.is_lt)
        sig_a = small.tile([128, n_pairs, NT], F32, tag="sig")
        sig_f = sig_a.rearrange("p a b -> p (a b)")
        nc.vector.select(sig_f, mk, pa, eb)

        s2 = small.tile([128, W], F32, tag="s2")
        nc.vector.tensor_mul(s2, sig_f, sig_f)
        ni_a = small.tile([128, n_pairs, NT], F32, tag="ni")
        ni_f = ni_a.rearrange("p a b -> p (a b)")
        nc.vector.reciprocal(ni_f, s2)
        nc.vector.tensor_scalar_mul(ni_f, ni_f, -0.5)  # -1/(2 sig^2)
        # c/sig as a DVE post-multiply (an exp bias of ln(c/sig) would
        # need the Ln table -> ~2.7us ACT table switches per swap)
        cs_a = small.tile([128, n_pairs, NT], F32, tag="cs")
        cs_f = cs_a.rearrange("p a b -> p (a b)")
        nc.vector.reciprocal(cs_f, sig_f)
        nc.vector.tensor_scalar_mul(cs_f, cs_f, _CNORM)

        vo_tiles = []
        for pr in range(n_pairs):
            qt = qk_pool.tile([E, L], BF16, tag="qt")
            nc.sync.dma_start(out=qt, in_=qT[pr])
            kt = qk_pool.tile([E, L], BF16, tag="kt")
            nc.sync.dma_start(out=kt, in_=kT[pr])
            vt = v_pool.tile([128, NT, E], BF16, tag="vt")
            nc.sync.dma_start(out=vt, in_=v[pr].rearrange("(j p) e -> p j e", p=128))
            # previous pair's V write: deferred one pair so this trigger
            # doesn't block the sig_bc/prior stream on the AV matmul chain
            if pr > 0:
                nc.gpsimd.dma_start(
                    out=oV[pr - 1].rearrange("(j p) e -> p j e", p=128),
                    in_=vo_tiles[pr - 1],
                )

            # --- sig broadcast [L] -> [L, S] (DVE compute; DMAs ride the
            # SWDGE queues to keep descriptor-gen off the sync sequencer) ---
            for j2 in range(NT // 2):
                gb = sgbc_pool.tile([128, 2, L], F32, tag="gb")
                for jj in range(2):
                    j = j2 * 2 + jj
                    nc.vector.tensor_scalar_mul(
                        gb[:, jj, :], ones_t, sig_a[:, pr, j : j + 1]
                    )
                nc.gpsimd.dma_start(
                    out=oG[pr, j2 * 256 : (j2 + 1) * 256, :].rearrange(
                        "(jj p) s -> p jj s", p=128
                    ),
                    in_=gb,
                )
            # --- prior band: (c/sig) * Exp(d2 * (-1/(2 sig^2))) ---
            for j in range(NT):
                vt_i = 0 if j == 0 else (2 if j == NT - 1 else 1)
                c0 = _band_c0(j)
                pb = pr_pool.tile([128, BAND], F32, tag="pb")
                nc.scalar.activation(
                    pb, d2b_t[:, vt_i, :], AF.Exp, scale=ni_a[:, pr, j : j + 1]
                )
                nc.vector.tensor_scalar_mul(pb, pb, cs_a[:, pr, j : j + 1])
                nc.gpsimd.dma_start(
                    out=oP[pr, j * 128 : (j + 1) * 128, c0 : c0 + BAND], in_=pb
                )
            # --- scores, exp, softmax denominators, series out ---
            # (2 row-tiles per SBUF tile -> 1MB DMAs, half the descriptor-gen
            # work on the sync sequencer)
            den = small.tile([128, NT], F32, tag="den")
            rden = small.tile([128, NT], F32, tag="rden")
            for j2 in range(NT // 2):
                es2 = ser_pool.tile([128, 2, L], F32, tag="es")
                for jj in range(2):
                    j = j2 * 2 + jj
                    ps = psum_sc.tile([128, L], F32, tag="scp")
                    nc.tensor.matmul(
                        ps[:, 0:512], qt[:, j * 128 : (j + 1) * 128], kt[:, 0:512],
                        start=True, stop=True,
                    )
                    nc.tensor.matmul(
                        ps[:, 512:1024], qt[:, j * 128 : (j + 1) * 128],
                        kt[:, 512:1024], start=True, stop=True,
                    )
                    nc.scalar.activation(
                        es2[:, jj, :], ps, AF.Exp, scale=_SCALE,
                        accum_out=den[:, j : j + 1],
                    )
                    nc.vector.reciprocal(rden[:, j : j + 1], den[:, j : j + 1])
                    nc.vector.tensor_scalar_mul(
                        es2[:, jj, :], es2[:, jj, :], rden[:, j : j + 1]
                    )
                nc.sync.dma_start(
                    out=oS[pr, j2 * 256 : (j2 + 1) * 256, :].rearrange(
                        "(jj p) s -> p jj s", p=128
                    ),
                    in_=es2,
                )

            # --- scores^T, exp (stationary operand for the AV matmul) ---
            exT = expT_pool.tile([128, NT, L], BF16, tag="exT")
            for i in range(NT):
                ps = psum_sc.tile([128, L], F32, tag="scp")
                nc.tensor.matmul(
                    ps[:, 0:512], kt[:, i * 128 : (i + 1) * 128], qt[:, 0:512],
                    start=True, stop=True,
                )
                nc.tensor.matmul(
                    ps[:, 512:1024], kt[:, i * 128 : (i + 1) * 128], qt[:, 512:1024],
                    start=True, stop=True,
                )
                nc.scalar.activation(exT[:, i, :], ps, AF.Exp, scale=_SCALE)

            # --- V = series @ values (accumulate over s-tiles), normalize.
            # The DMA is deferred to after the pair loop: a V write here would
            # sit in the gpsimd sequencer program ahead of the next pair's
            # sig_bc/prior triggers and stall that whole DMA stream on the
            # AV matmul chain. ---
            vo = vout_pool.tile([128, NT, E], F32, tag="vo")
            vo_tiles.append(vo)
            for j in range(NT):
                pv = psum_v.tile([128, E], F32, tag="pv")
                for i in range(NT):
                    nc.tensor.matmul(
                        pv,
                        exT[:, i, j * 128 : (j + 1) * 128],
                        vt[:, i, :],
                        start=(i == 0),
                        stop=(i == NT - 1),
                    )
                nc.vector.tensor_scalar_mul(vo[:, j, :], pv, rden[:, j : j + 1])



        nc.gpsimd.dma_start(
            out=oV[n_pairs - 1].rearrange("(j p) e -> p j e", p=128),
            in_=vo_tiles[-1],
        )

    nc.compile()
    return nc


_NC = None


def _get_nc():
    global _NC
    if _NC is None:
        _NC = build_nc()
    return _NC


def _pairs(core):
    return [(f // H, f % H) for f in range(core * NPAIR, (core + 1) * NPAIR)]


def _make_in_maps(inputs):
    from ml_dtypes import bfloat16

    queries = np.asarray(inputs["queries"], dtype=np.float32)
    keys = np.asarray(inputs["keys"], dtype=np.float32)
    values = np.asarray(inputs["values"], dtype=np.float32)
    sigma = np.asarray(inputs["sigma"], dtype=np.float32)

    d2b = _d2_band()
    in_maps = []
    for c in range(N_CORES):
        prs = _pairs(c)
        in_maps.append(
            {
                "qT": np.stack(
                    [queries[b, :, h, :].T for b, h in prs]
                ).astype(bfloat16),
                "kT": np.stack([keys[b, :, h, :].T for b, h in prs]).astype(bfloat16),
                "v": np.stack([values[b, :, h, :] for b, h in prs]).astype(bfloat16),
                "sg": np.stack(
                    [sigma[b, :, h].reshape(NT, 128).T for b, h in prs], axis=1
                ).copy(),
                "d2b": d2b,
            }
        )
    return in_maps


def kernel(queries, keys, values, sigma):
    nc = _get_nc()
    in_maps = _make_in_maps(
        {"queries": queries, "keys": keys, "values": values, "sigma": sigma}
    )

    res = run_bass_kernel_spmd(nc, in_maps, core_ids=list(range(N_CORES)))

    V = np.empty((B, L, H, E), np.float32)
    series = np.empty((B, H, L, L), np.float32)
    prior = np.empty((B, H, L, L), np.float32)
    sigbc = np.empty((B, H, L, L), np.float32)
    for c in range(N_CORES):
        r = res.results[c]
        for idx, (b, h) in enumerate(_pairs(c)):
            V[b, :, h, :] = r["oV"][idx]
            series[b, h] = r["oS"][idx]
            prior[b, h] = r["oP"][idx]
            sigbc[b, h] = r["oG"][idx]
    return V, series, prior, sigbc
